# revision 9
# baseline (speedup 1.0000x reference)
"""Binary conv (BN -> sign -> binarized 3x3 conv -> bias -> relu) on 8 TRN2 cores.

Strategy (v2)
-------------
Data-parallel over batch: each of the 8 NeuronCores gets 8 of the 64 images.

  phase P (prologue):  warm-up AllReduce (wakes ncfw so the real one has a
                       short trigger latency); load w, sign() -> bf16,
                       PE-transpose each [co,ci] 128x128 block into lhsT
                       layout, store as fp8e4 [ci, tap, co_chunk, j, co].
  phase A (stats):     stream x shard as 16 tiles x 2 half-DMAs, alternating
                       between the sync and scalar HWDGE queues (two physical
                       rings -> closer to the 358 GB/s HBM cap). DVE
                       reduce_sum per half, ScalarE Square+accum_out per
                       tile. One [128,4] fp32 AllReduce across the 8 cores;
                       then scale_c = gamma_c * rsqrt(var_c+eps),
                       shift_c = beta_c - mean_c*scale_c.
  phase B (conv):      x is re-streamed (sync queue; the first images'
                       loads are queued right behind phase A so they prefetch
                       during the AllReduce gap). Per image: ACT computes
                       sign(scale*x + shift) -> fp8e4 into a zero-padded flat
                       [58*58] SBUF plane (3 planes rotating); conv as 9 taps
                       x fp8 DoubleRow matmuls (contracting all 256 ci at
                       once) into [128co x 8x56] PSUM tiles (4-dim moving AP
                       skips the 2 wrap columns entirely); DVE fuses +bias
                       and relu on the PSUM evacuation; DMA out on sync.

sign() outputs +-1 exactly representable in fp8e4, PE accumulates in fp32
(integer sums bounded by 2304), so the conv arithmetic is exact.
"""

import os
import sys

import numpy as np

for _p in ("/opt/trn_rl_repo", "/root/.axon_site/_ro/trn_rl_repo"):
    if os.path.isdir(_p) and _p not in sys.path:
        sys.path.append(_p)

import concourse.bass as bass
import concourse.bacc as bacc
import concourse.tile as tile
from concourse import mybir
from concourse.bass_utils import run_bass_kernel_spmd
from concourse.masks import make_identity

AF = mybir.ActivationFunctionType
ALU = mybir.AluOpType
F32 = mybir.dt.float32
BF16 = mybir.dt.bfloat16
FP8 = mybir.dt.float8e4
DR = mybir.MatmulPerfMode.DoubleRow

N_CORES = 8
N_IMG = 8          # images per core
C = 256            # channels (in == out)
H = W = 56
HW = H * W         # 3136
HALF = HW // 2     # 1568
PW = W + 2         # 58 padded
PLANE = PW * PW    # 3364
# plane data at offset 1 (guard elem before); padded to 3376 so the DoubleRow
# pair stride (PLANE_G fp8 elements) is a multiple of 16
PLANE_G = 3376
EPS = 1e-5
N_TOTAL = 64 * HW  # BN reduction count over full batch
ROWS_PER_BLK = 8
N_BLK = H // ROWS_PER_BLK        # 7
OUT_FREE = ROWS_PER_BLK * W      # 448 valid outputs per block
N_PLANES = 3       # rotating sign planes (sign can run 2 images ahead)

_CACHE = {}


def _build_nc():
    nc = bacc.Bacc(None, target_bir_lowering=False, num_devices=N_CORES)

    x_d = nc.dram_tensor("x", [N_IMG, C, HW], F32, kind="ExternalInput")
    g_d = nc.dram_tensor("gamma", [C], F32, kind="ExternalInput")
    be_d = nc.dram_tensor("beta", [C], F32, kind="ExternalInput")
    w_d = nc.dram_tensor("w", [C, C * 9], F32, kind="ExternalInput")
    b_d = nc.dram_tensor("b", [C], F32, kind="ExternalInput")
    y_d = nc.dram_tensor("y", [N_IMG, C, HW], F32, kind="ExternalOutput")
    # single [128,4] stats AllReduce (ncfw serializes queued collectives and
    # each pays a skew-dominated rendezvous, so splitting per-chunk is a loss)
    cc_in = nc.dram_tensor("cc_in", [128, 4], F32)
    cc_out = nc.dram_tensor("cc_out", [128, 4], F32, addr_space="Shared")
    # tiny warm-up collective: wakes the ncfw collective path early (and eats
    # the first-collective setup cost) so the real AllReduce is fast
    wm_in = nc.dram_tensor("wm_in", [1, 1], F32)
    wm_out = nc.dram_tensor("wm_out", [1, 1], F32, addr_space="Shared")

    with tile.TileContext(nc) as tc:
        with (
            tc.tile_pool(name="persist", bufs=1) as persist,
            tc.tile_pool(name="xin", bufs=10) as xin_pool,     # x staging, both phases
            tc.tile_pool(name="wpre", bufs=1) as wpre_pool,    # w staging
            tc.tile_pool(name="trash", bufs=1) as trash_pool,
            tc.tile_pool(name="outp", bufs=4) as out_pool,
            tc.tile_pool(name="vec", bufs=1) as vec_pool,
        ):
            # padded+binarized activation planes, rotating over 3 buffers so
            # sign() for image n+1/n+2 doesn't WAR-serialize on conv reads:
            # [ci_part, ci_pair(j), guarded flat plane]
            xpads = [
                persist.tile([128, 2, PLANE_G], FP8, name=f"xpad{i}")
                for i in range(N_PLANES)
            ]
            # conv weights, fp8 DoubleRow lhsT layout: [ci_part, tap, co_chunk, j, co]
            wt = persist.tile([128, 9, 2, 2, 128], FP8)

            # ---- tiny setup on the gpsimd queue (order matters: the weight
            # DMAs must reach the queue before the big plane memsets) ----
            wm_sb = vec_pool.tile([1, 1], F32)
            nc.gpsimd.memset(wm_sb, 0.0)
            nc.gpsimd.dma_start(wm_in[:], wm_sb)
            nc.gpsimd.collective_compute(
                "AllReduce",
                ALU.add,
                replica_groups=[list(range(N_CORES))],
                ins=[wm_in[:]],
                outs=[wm_out[:]],
            )

            # per-channel vectors, [128, 2] = (partition, ci_chunk)
            gamma_sb = vec_pool.tile([128, 2], F32)
            beta_sb = vec_pool.tile([128, 2], F32)
            bias_sb = vec_pool.tile([128, 2], F32)
            nc.gpsimd.dma_start(gamma_sb, g_d.rearrange("(c p) -> p c", p=128))
            nc.gpsimd.dma_start(beta_sb, be_d.rearrange("(c p) -> p c", p=128))
            nc.gpsimd.dma_start(bias_sb, b_d.rearrange("(c p) -> p c", p=128))

            # ---------------- phase A x stream: 16 tiles x 2 half-DMAs.
            # Even halves go to the sync HWDGE ring (all dispatches up front;
            # nothing else contends there in phase A). Odd halves go to the
            # scalar HWDGE ring with a 3-tile lookahead, paced by the squares:
            # a deep dispatch backlog on the ACT queue head-of-line-blocks the
            # compute behind it when the HW ring fills up.
            N_TILES = 2 * N_IMG  # (n, c) pairs, n-major
            sums = vec_pool.tile([128, 2, 2 * N_IMG], F32)   # per half
            sumsq = vec_pool.tile([128, 2, N_IMG], F32)      # per tile
            cc_sb = vec_pool.tile([128, 2, 2], F32)          # per chunk: (sum, sumsq)

            xa_tiles = []

            def emit_xa_tile(idx):
                n, c = divmod(idx, 2)
                xt = xin_pool.tile([128, HW], F32, name=f"xa{idx}", tag="x")
                xa_tiles.append(xt)
                nc.sync.dma_start(
                    xt[:, :HALF], x_d[n, c * 128 : (c + 1) * 128, :HALF]
                )
                return xt

            def emit_xa_odd(idx):
                n, c = divmod(idx, 2)
                nc.scalar.dma_start(
                    xa_tiles[idx][:, HALF:],
                    x_d[n, c * 128 : (c + 1) * 128, HALF:],
                )

            for idx in range(N_TILES):
                emit_xa_tile(idx)
            for idx in range(3):
                emit_xa_odd(idx)

            # ---------------- phase P: weights (own staging pool; gpsimd DMA
            # + ACT sign + PE transposes + DVE copies, all during the stream)
            ident = vec_pool.tile([128, 128], BF16)
            make_identity(nc, ident)
            ws = wpre_pool.tile([128, 2, C * 9], BF16, bufs=1)
            with tc.tile_pool(name="wps", bufs=2, space="PSUM") as wps:
                for o in range(2):
                    wf = wpre_pool.tile([128, C * 9], F32, bufs=2)
                    nc.gpsimd.dma_start(wf, w_d[o * 128 : (o + 1) * 128, :])
                    nc.scalar.activation(ws[:, o, :], wf, AF.Sign)
                ws_r = ws.rearrange("p o (ci tap) -> p o ci tap", tap=9)
                for t in range(9):
                    for c in range(2):
                        for o in range(2):
                            pw = wps.tile([128, 128], BF16)
                            nc.tensor.transpose(
                                pw, ws_r[:, o, c * 128 : (c + 1) * 128, t], ident
                            )
                            nc.vector.tensor_copy(wt[:, t, o, c, :], pw)

            # zero the sign planes once (borders + guards stay zero; sign only
            # ever writes the interior) -- on the otherwise idle gpsimd engine
            for xp in xpads:
                nc.gpsimd.memset(xp.rearrange("p a b -> p (a b)"), 0.0)

            # ---------------- phase A reductions (fire per half/tile as the
            # DMAs land; stats complete ~3us after the last chunk arrives)
            for idx in range(N_TILES):
                n, c = divmod(idx, 2)
                xt = xa_tiles[idx]
                for h in range(2):
                    nc.vector.reduce_sum(
                        sums[:, c, n * 2 + h : n * 2 + h + 1],
                        xt[:, h * HALF : (h + 1) * HALF],
                        axis=mybir.AxisListType.X,
                    )
                tr = trash_pool.tile([128, HW], F32)
                nc.scalar.activation(
                    tr, xt, AF.Square, accum_out=sumsq[:, c, n : n + 1]
                )
                if idx + 3 < N_TILES:
                    emit_xa_odd(idx + 3)

            for c in range(2):
                nc.vector.reduce_sum(
                    cc_sb[:, c, 0:1], sums[:, c, :], axis=mybir.AxisListType.X
                )
                nc.vector.reduce_sum(
                    cc_sb[:, c, 1:2], sumsq[:, c, :], axis=mybir.AxisListType.X
                )

            nc.gpsimd.dma_start(cc_in[:], cc_sb.rearrange("p a b -> p (a b)"))
            nc.gpsimd.collective_compute(
                "AllReduce",
                ALU.add,
                replica_groups=[list(range(N_CORES))],
                ins=[cc_in[:]],
                outs=[cc_out[:]],
            )

            gl = vec_pool.tile([128, 2, 2], F32)
            nc.gpsimd.dma_start(gl.rearrange("p a b -> p (a b)"), cc_out[:])

            # per-chunk finalize: scale_c = gamma_c / sqrt(var_c + eps),
            # shift_c = beta_c - mean_c * scale_c. Abs_reciprocal_sqrt's loose
            # precision only scales scl's magnitude (scl stays > 0), which
            # sign() cannot observe — outputs remain exact.
            eps_sb = vec_pool.tile([128, 1], F32)
            nc.vector.memset(eps_sb, EPS)
            mean = vec_pool.tile([128, 2], F32)
            m2 = vec_pool.tile([128, 2], F32)
            var = vec_pool.tile([128, 2], F32)
            rstd = vec_pool.tile([128, 2], F32)
            scl = vec_pool.tile([128, 2], F32)
            sh = vec_pool.tile([128, 2], F32)
            for c in range(2):
                cs = slice(c, c + 1)
                nc.vector.tensor_scalar_mul(
                    mean[:, cs], gl[:, c, 0:1], 1.0 / N_TOTAL
                )
                nc.vector.tensor_tensor(
                    m2[:, cs], mean[:, cs], mean[:, cs], op=ALU.mult
                )
                nc.vector.scalar_tensor_tensor(
                    out=var[:, cs],
                    in0=gl[:, c, 1:2],
                    scalar=1.0 / N_TOTAL,
                    in1=m2[:, cs],
                    op0=ALU.mult,
                    op1=ALU.subtract,
                )
                nc.scalar.activation(
                    rstd[:, cs], var[:, cs], AF.Abs_reciprocal_sqrt,
                    bias=eps_sb[:],
                )
                nc.vector.tensor_mul(scl[:, cs], gamma_sb[:, cs], rstd[:, cs])
                nc.vector.tensor_mul(sh[:, cs], mean[:, cs], scl[:, cs])
                nc.vector.tensor_sub(sh[:, cs], beta_sb[:, cs], sh[:, cs])

            # ---------------- phase B: sign + conv ----------------
            # x reloads ride the sync queue right behind phase A's stream, so
            # the first images prefetch for free during the AllReduce gap.
            xb_tiles = {}

            def emit_xb_dma(n):
                for c in range(2):
                    xt = xin_pool.tile([128, HW], F32, name=f"xb{n}_{c}", tag="x")
                    xb_tiles[(n, c)] = xt
                    nc.sync.dma_start(xt, x_d[n, c * 128 : (c + 1) * 128, :])

            def emit_sign(n):
                xp = xpads[n % N_PLANES]
                xrow = xp[:, :, 1 : 1 + PLANE].rearrange(
                    "p j (r w) -> p j r w", w=PW
                )
                for c in range(2):
                    nc.scalar.activation(
                        xrow[:, c, 1 : H + 1, 1 : W + 1],
                        xb_tiles.pop((n, c)).rearrange("p (h w) -> p h w", w=W),
                        AF.Sign,
                        bias=sh[:, c : c + 1],
                        scale=scl[:, c : c + 1],
                    )

            def emit_conv(n, cps):
                xp = xpads[n % N_PLANES]
                for o in range(2):
                    for bi in range(N_BLK):
                        ps = cps.tile([128, OUT_FREE], F32)
                        r0 = bi * ROWS_PER_BLK
                        for t in range(9):
                            ky, kx = divmod(t, 3)
                            s = 1 + (r0 + ky) * PW + kx
                            xr = xp[:, :, s : s + ROWS_PER_BLK * PW].rearrange(
                                "p j (r w) -> p j r w", w=PW
                            )
                            nc.tensor.matmul(
                                ps,
                                wt[:, t, o],
                                xr[:, :, :, 0:W],
                                start=(t == 0),
                                stop=(t == 8),
                                perf_mode=DR,
                            )
                        ob = out_pool.tile([128, OUT_FREE], F32)
                        # relu(psum + bias): (x + b) then max(.., 0) on DVE
                        nc.vector.tensor_scalar(
                            out=ob,
                            in0=ps,
                            scalar1=bias_sb[:, o : o + 1],
                            scalar2=0.0,
                            op0=ALU.add,
                            op1=ALU.max,
                        )
                        nc.sync.dma_start(
                            y_d[
                                n, o * 128 : (o + 1) * 128,
                                bi * OUT_FREE : (bi + 1) * OUT_FREE,
                            ],
                            ob,
                        )

            with tc.tile_pool(name="cps", bufs=8, space="PSUM") as cps:
                emit_xb_dma(0)
                emit_xb_dma(1)
                for n in range(N_IMG):
                    emit_sign(n)
                    if n + 2 < N_IMG:
                        emit_xb_dma(n + 2)
                    if n >= 1:
                        emit_conv(n - 1, cps)
                emit_conv(N_IMG - 1, cps)

    nc.finalize()
    return nc


def get_nc():
    if "nc" not in _CACHE:
        _CACHE["nc"] = _build_nc()
    return _CACHE["nc"]


def run(x, gamma, beta, w, b, trace=False, trace_cores=None):
    x = np.ascontiguousarray(np.asarray(x, dtype=np.float32))
    gamma = np.ascontiguousarray(np.asarray(gamma, dtype=np.float32))
    beta = np.ascontiguousarray(np.asarray(beta, dtype=np.float32))
    w = np.ascontiguousarray(np.asarray(w, dtype=np.float32)).reshape(C, C * 9)
    b = np.ascontiguousarray(np.asarray(b, dtype=np.float32))

    nc = get_nc()
    in_maps = []
    for i in range(N_CORES):
        in_maps.append(
            {
                "x": np.ascontiguousarray(
                    x[i * N_IMG : (i + 1) * N_IMG].reshape(N_IMG, C, HW)
                ),
                "gamma": gamma,
                "beta": beta,
                "w": w,
                "b": b,
            }
        )
    res = run_bass_kernel_spmd(
        nc, in_maps, list(range(N_CORES)), trace=trace, trace_cores=trace_cores
    )
    y = np.concatenate(
        [r["y"].reshape(N_IMG, C, H, W) for r in res.results], axis=0
    )
    return y.astype(np.float32), res


def kernel(x, gamma, beta, w, b):
    y, _ = run(x, gamma, beta, w, b, trace=False)
    return y


# revision 15
# speedup vs baseline: 1.0458x; 1.0458x over previous
"""Binary conv (BN -> sign -> binarized 3x3 conv -> bias -> relu) on 8 TRN2 cores.

Strategy (v2)
-------------
Data-parallel over batch: each of the 8 NeuronCores gets 8 of the 64 images.

  phase P (prologue):  warm-up AllReduce (wakes ncfw so the real one has a
                       short trigger latency); load w, sign() -> bf16,
                       PE-transpose each [co,ci] 128x128 block into lhsT
                       layout, store as fp8e4 [ci, tap, co_chunk, j, co].
  phase A (stats):     stream x shard as 16 tiles x 2 half-DMAs, alternating
                       between the sync and scalar HWDGE queues (two physical
                       rings -> closer to the 358 GB/s HBM cap). DVE
                       reduce_sum per half, ScalarE Square+accum_out per
                       tile. One [128,4] fp32 AllReduce across the 8 cores;
                       then scale_c = gamma_c * rsqrt(var_c+eps),
                       shift_c = beta_c - mean_c*scale_c.
  phase B (conv):      x is re-streamed (sync queue; the first images'
                       loads are queued right behind phase A so they prefetch
                       during the AllReduce gap). Per image: ACT computes
                       sign(scale*x + shift) -> fp8e4 into a zero-padded flat
                       [58*58] SBUF plane (3 planes rotating); conv as 9 taps
                       x fp8 DoubleRow matmuls (contracting all 256 ci at
                       once) into [128co x 8x56] PSUM tiles (4-dim moving AP
                       skips the 2 wrap columns entirely); DVE fuses +bias
                       and relu on the PSUM evacuation; DMA out on sync.

sign() outputs +-1 exactly representable in fp8e4, PE accumulates in fp32
(integer sums bounded by 2304), so the conv arithmetic is exact.
"""

import os
import sys

import numpy as np

for _p in ("/opt/trn_rl_repo", "/root/.axon_site/_ro/trn_rl_repo"):
    if os.path.isdir(_p) and _p not in sys.path:
        sys.path.append(_p)

import concourse.bass as bass
import concourse.bacc as bacc
import concourse.tile as tile
from concourse import mybir
from concourse.bass_utils import run_bass_kernel_spmd
from concourse.masks import make_identity

AF = mybir.ActivationFunctionType
ALU = mybir.AluOpType
F32 = mybir.dt.float32
BF16 = mybir.dt.bfloat16
FP8 = mybir.dt.float8e4
DR = mybir.MatmulPerfMode.DoubleRow

N_CORES = 8
N_IMG = 8          # images per core
C = 256            # channels (in == out)
H = W = 56
HW = H * W         # 3136
HALF = HW // 2     # 1568
PW = W + 2         # 58 padded
PLANE = PW * PW    # 3364
# plane data at offset 1 (guard elem before); padded to 3376 so the DoubleRow
# pair stride (PLANE_G fp8 elements) is a multiple of 16
PLANE_G = 3376
EPS = 1e-5
N_TOTAL = 64 * HW  # BN reduction count over full batch
ROWS_PER_BLK = 8
N_BLK = H // ROWS_PER_BLK        # 7
BLK_FREE = ROWS_PER_BLK * PW     # 464 px per matmul (incl. 2 wrap cols/row;
                                 # a flat AP streams ~20% faster than a 4-dim
                                 # AP that skips them: row-transition overhead)
OUT_FREE = ROWS_PER_BLK * W      # 448 valid outputs per block
N_PLANES = 3       # rotating sign planes (sign can run 2 images ahead)

_CACHE = {}


def _build_nc():
    nc = bacc.Bacc(None, target_bir_lowering=False, num_devices=N_CORES)

    x_d = nc.dram_tensor("x", [N_IMG, C, HW], F32, kind="ExternalInput")
    g_d = nc.dram_tensor("gamma", [C], F32, kind="ExternalInput")
    be_d = nc.dram_tensor("beta", [C], F32, kind="ExternalInput")
    w_d = nc.dram_tensor("w", [C, C * 9], F32, kind="ExternalInput")
    b_d = nc.dram_tensor("b", [C], F32, kind="ExternalInput")
    y_d = nc.dram_tensor("y", [N_IMG, C, HW], F32, kind="ExternalOutput")
    # single [128,4] stats AllReduce (ncfw serializes queued collectives and
    # each pays a skew-dominated rendezvous, so splitting per-chunk is a loss)
    cc_in = nc.dram_tensor("cc_in", [128, 4], F32)
    cc_out = nc.dram_tensor("cc_out", [128, 4], F32, addr_space="Shared")
    # tiny warm-up collective: wakes the ncfw collective path early (and eats
    # the first-collective setup cost) so the real AllReduce is fast
    wm_in = nc.dram_tensor("wm_in", [1, 1], F32)
    wm_out = nc.dram_tensor("wm_out", [1, 1], F32, addr_space="Shared")

    with tile.TileContext(nc) as tc:
        with (
            tc.tile_pool(name="persist", bufs=1) as persist,
            tc.tile_pool(name="xin", bufs=10) as xin_pool,     # x staging, both phases
            tc.tile_pool(name="wpre", bufs=1) as wpre_pool,    # w staging
            tc.tile_pool(name="trash", bufs=1) as trash_pool,
            tc.tile_pool(name="outp", bufs=4) as out_pool,
            tc.tile_pool(name="vec", bufs=1) as vec_pool,
        ):
            # padded+binarized activation planes, rotating over 3 buffers so
            # sign() for image n+1/n+2 doesn't WAR-serialize on conv reads:
            # [ci_part, ci_pair(j), guarded flat plane]
            xpads = [
                persist.tile([128, 2, PLANE_G], FP8, name=f"xpad{i}")
                for i in range(N_PLANES)
            ]
            # conv weights, fp8 DoubleRow lhsT layout: [ci_part, tap, co_chunk, j, co]
            wt = persist.tile([128, 9, 2, 2, 128], FP8)

            # ---- tiny setup on the gpsimd queue (order matters: the weight
            # DMAs must reach the queue before the big plane memsets) ----
            wm_sb = vec_pool.tile([1, 1], F32)
            nc.gpsimd.memset(wm_sb, 0.0)
            nc.gpsimd.dma_start(wm_in[:], wm_sb)
            nc.gpsimd.collective_compute(
                "AllReduce",
                ALU.add,
                replica_groups=[list(range(N_CORES))],
                ins=[wm_in[:]],
                outs=[wm_out[:]],
            )

            # per-channel vectors, [128, 2] = (partition, ci_chunk)
            gamma_sb = vec_pool.tile([128, 2], F32)
            beta_sb = vec_pool.tile([128, 2], F32)
            bias_sb = vec_pool.tile([128, 2], F32)
            nc.gpsimd.dma_start(gamma_sb, g_d.rearrange("(c p) -> p c", p=128))
            nc.gpsimd.dma_start(beta_sb, be_d.rearrange("(c p) -> p c", p=128))
            nc.gpsimd.dma_start(bias_sb, b_d.rearrange("(c p) -> p c", p=128))

            # ---------------- phase A x stream: 16 tiles x 2 half-DMAs.
            # Even halves go to the sync HWDGE ring (all dispatches up front;
            # nothing else contends there in phase A). Odd halves go to the
            # scalar HWDGE ring with a 3-tile lookahead, paced by the squares:
            # a deep dispatch backlog on the ACT queue head-of-line-blocks the
            # compute behind it when the HW ring fills up.
            N_TILES = 2 * N_IMG  # (n, c) pairs, n-major
            sums = vec_pool.tile([128, 2, 2 * N_IMG], F32)   # per half
            sumsq = vec_pool.tile([128, 2, N_IMG], F32)      # per tile
            cc_sb = vec_pool.tile([128, 2, 2], F32)          # per chunk: (sum, sumsq)

            xa_tiles = []

            def emit_xa_tile(idx):
                n, c = divmod(idx, 2)
                xt = xin_pool.tile([128, HW], F32, name=f"xa{idx}", tag="x")
                xa_tiles.append(xt)
                nc.sync.dma_start(
                    xt[:, :HALF], x_d[n, c * 128 : (c + 1) * 128, :HALF]
                )
                return xt

            def emit_xa_odd(idx):
                n, c = divmod(idx, 2)
                nc.scalar.dma_start(
                    xa_tiles[idx][:, HALF:],
                    x_d[n, c * 128 : (c + 1) * 128, HALF:],
                )

            for idx in range(N_TILES):
                emit_xa_tile(idx)
            for idx in range(3):
                emit_xa_odd(idx)

            # ---------------- phase P: weights (own staging pool; gpsimd DMA
            # + ACT sign + PE transposes + DVE copies, all during the stream)
            ident = vec_pool.tile([128, 128], BF16)
            make_identity(nc, ident)
            ws = wpre_pool.tile([128, 2, C * 9], BF16, bufs=1)
            with tc.tile_pool(name="wps", bufs=2, space="PSUM") as wps:
                for o in range(2):
                    wf = wpre_pool.tile([128, C * 9], F32, bufs=2)
                    nc.gpsimd.dma_start(wf, w_d[o * 128 : (o + 1) * 128, :])
                    nc.scalar.activation(ws[:, o, :], wf, AF.Sign)
                ws_r = ws.rearrange("p o (ci tap) -> p o ci tap", tap=9)
                for t in range(9):
                    for c in range(2):
                        for o in range(2):
                            pw = wps.tile([128, 128], BF16)
                            nc.tensor.transpose(
                                pw, ws_r[:, o, c * 128 : (c + 1) * 128, t], ident
                            )
                            nc.vector.tensor_copy(wt[:, t, o, c, :], pw)

            # zero the sign planes once (borders + guards stay zero; sign only
            # ever writes the interior) -- on the otherwise idle gpsimd engine
            for xp in xpads:
                nc.gpsimd.memset(xp.rearrange("p a b -> p (a b)"), 0.0)

            # ---------------- phase A reductions (fire per half/tile as the
            # DMAs land; stats complete ~3us after the last chunk arrives)
            for idx in range(N_TILES):
                n, c = divmod(idx, 2)
                xt = xa_tiles[idx]
                for h in range(2):
                    nc.vector.reduce_sum(
                        sums[:, c, n * 2 + h : n * 2 + h + 1],
                        xt[:, h * HALF : (h + 1) * HALF],
                        axis=mybir.AxisListType.X,
                    )
                tr = trash_pool.tile([128, HW], F32)
                nc.scalar.activation(
                    tr, xt, AF.Square, accum_out=sumsq[:, c, n : n + 1]
                )
                if idx + 3 < N_TILES:
                    emit_xa_odd(idx + 3)

            for c in range(2):
                nc.vector.reduce_sum(
                    cc_sb[:, c, 0:1], sums[:, c, :], axis=mybir.AxisListType.X
                )
                nc.vector.reduce_sum(
                    cc_sb[:, c, 1:2], sumsq[:, c, :], axis=mybir.AxisListType.X
                )

            nc.gpsimd.dma_start(cc_in[:], cc_sb.rearrange("p a b -> p (a b)"))
            nc.gpsimd.collective_compute(
                "AllReduce",
                ALU.add,
                replica_groups=[list(range(N_CORES))],
                ins=[cc_in[:]],
                outs=[cc_out[:]],
            )

            gl = vec_pool.tile([128, 2, 2], F32)
            nc.gpsimd.dma_start(gl.rearrange("p a b -> p (a b)"), cc_out[:])

            # phase-B x reloads go on the scalar ring, gated on the stats
            # being done (the gate DMA reads cc_in, which is written
            # post-consolidation): prefetch fills exactly the AllReduce gap
            # and cannot steal HBM bandwidth from the stats-critical phase A
            # stream. Emitted BEFORE the finalize block so the prefetch
            # dispatches sit ahead of the AR-dependent rstd ops in the ACT
            # queue.
            gate_sb = vec_pool.tile([128, 4], F32)
            nc.scalar.dma_start(gate_sb, cc_in[:])

            xb_tiles = {}

            def emit_xb_dma(n):
                for c in range(2):
                    xt = xin_pool.tile([128, HW], F32, name=f"xb{n}_{c}", tag="x")
                    xb_tiles[(n, c)] = xt
                    nc.scalar.dma_start(xt, x_d[n, c * 128 : (c + 1) * 128, :])

            emit_xb_dma(0)
            emit_xb_dma(1)
            emit_xb_dma(2)

            # per-chunk finalize: scale_c = gamma_c / sqrt(var_c + eps),
            # shift_c = beta_c - mean_c * scale_c. Abs_reciprocal_sqrt's loose
            # precision only scales scl's magnitude (scl stays > 0), which
            # sign() cannot observe — outputs remain exact.
            eps_sb = vec_pool.tile([128, 1], F32)
            nc.vector.memset(eps_sb, EPS)
            mean = vec_pool.tile([128, 2], F32)
            m2 = vec_pool.tile([128, 2], F32)
            var = vec_pool.tile([128, 2], F32)
            rstd = vec_pool.tile([128, 2], F32)
            scl = vec_pool.tile([128, 2], F32)
            sh = vec_pool.tile([128, 2], F32)
            for c in range(2):
                cs = slice(c, c + 1)
                nc.vector.tensor_scalar_mul(
                    mean[:, cs], gl[:, c, 0:1], 1.0 / N_TOTAL
                )
                nc.vector.tensor_tensor(
                    m2[:, cs], mean[:, cs], mean[:, cs], op=ALU.mult
                )
                nc.vector.scalar_tensor_tensor(
                    out=var[:, cs],
                    in0=gl[:, c, 1:2],
                    scalar=1.0 / N_TOTAL,
                    in1=m2[:, cs],
                    op0=ALU.mult,
                    op1=ALU.subtract,
                )
                nc.scalar.activation(
                    rstd[:, cs], var[:, cs], AF.Abs_reciprocal_sqrt,
                    bias=eps_sb[:],
                )
                nc.vector.tensor_mul(scl[:, cs], gamma_sb[:, cs], rstd[:, cs])
                nc.vector.tensor_mul(sh[:, cs], mean[:, cs], scl[:, cs])
                nc.vector.tensor_sub(sh[:, cs], beta_sb[:, cs], sh[:, cs])

            # ---------------- phase B: sign + conv ----------------
            def emit_sign(n):
                xp = xpads[n % N_PLANES]
                xrow = xp[:, :, 1 : 1 + PLANE].rearrange(
                    "p j (r w) -> p j r w", w=PW
                )
                for c in range(2):
                    nc.scalar.activation(
                        xrow[:, c, 1 : H + 1, 1 : W + 1],
                        xb_tiles.pop((n, c)).rearrange("p (h w) -> p h w", w=W),
                        AF.Sign,
                        bias=sh[:, c : c + 1],
                        scale=scl[:, c : c + 1],
                    )

            def emit_conv(n, cps):
                xp = xpads[n % N_PLANES]
                for o in range(2):
                    for bi in range(N_BLK):
                        ps = cps.tile([128, BLK_FREE], F32)
                        r0 = bi * ROWS_PER_BLK
                        for t in range(9):
                            ky, kx = divmod(t, 3)
                            s = 1 + (r0 + ky) * PW + (kx - 1)
                            nc.tensor.matmul(
                                ps,
                                wt[:, t, o],
                                xp[:, :, s : s + BLK_FREE],
                                start=(t == 0),
                                stop=(t == 8),
                                perf_mode=DR,
                            )
                        ob = out_pool.tile([128, OUT_FREE], F32)
                        # relu(psum + bias): (x + b) then max(.., 0) on DVE,
                        # dropping the 2 wrap columns of each row
                        nc.vector.tensor_scalar(
                            out=ob,
                            in0=ps.rearrange("p (r c) -> p r c", c=PW)[
                                :, :, 1 : W + 1
                            ],
                            scalar1=bias_sb[:, o : o + 1],
                            scalar2=0.0,
                            op0=ALU.add,
                            op1=ALU.max,
                        )
                        nc.sync.dma_start(
                            y_d[
                                n, o * 128 : (o + 1) * 128,
                                bi * OUT_FREE : (bi + 1) * OUT_FREE,
                            ],
                            ob,
                        )

            with tc.tile_pool(name="cps", bufs=8, space="PSUM") as cps:
                for n in range(N_IMG):
                    emit_sign(n)
                    if n + 3 < N_IMG:
                        emit_xb_dma(n + 3)
                    if n >= 1:
                        emit_conv(n - 1, cps)
                emit_conv(N_IMG - 1, cps)

    nc.finalize()
    return nc


def get_nc():
    if "nc" not in _CACHE:
        _CACHE["nc"] = _build_nc()
    return _CACHE["nc"]


def run(x, gamma, beta, w, b, trace=False, trace_cores=None):
    x = np.ascontiguousarray(np.asarray(x, dtype=np.float32))
    gamma = np.ascontiguousarray(np.asarray(gamma, dtype=np.float32))
    beta = np.ascontiguousarray(np.asarray(beta, dtype=np.float32))
    w = np.ascontiguousarray(np.asarray(w, dtype=np.float32)).reshape(C, C * 9)
    b = np.ascontiguousarray(np.asarray(b, dtype=np.float32))

    nc = get_nc()
    in_maps = []
    for i in range(N_CORES):
        in_maps.append(
            {
                "x": np.ascontiguousarray(
                    x[i * N_IMG : (i + 1) * N_IMG].reshape(N_IMG, C, HW)
                ),
                "gamma": gamma,
                "beta": beta,
                "w": w,
                "b": b,
            }
        )
    res = run_bass_kernel_spmd(
        nc, in_maps, list(range(N_CORES)), trace=trace, trace_cores=trace_cores
    )
    y = np.concatenate(
        [r["y"].reshape(N_IMG, C, H, W) for r in res.results], axis=0
    )
    return y.astype(np.float32), res


def kernel(x, gamma, beta, w, b):
    y, _ = run(x, gamma, beta, w, b, trace=False)
    return y


# revision 18
# speedup vs baseline: 1.1341x; 1.0844x over previous
"""Binary conv (BN -> sign -> binarized 3x3 conv -> bias -> relu) on 8 TRN2 cores.

Strategy (v2)
-------------
Data-parallel over batch: each of the 8 NeuronCores gets 8 of the 64 images.

  phase P (prologue):  warm-up AllReduce (wakes ncfw so the real one has a
                       short trigger latency); load w, sign() -> bf16,
                       PE-transpose each [co,ci] 128x128 block into lhsT
                       layout, store as fp8e4 [ci, tap, co_chunk, j, co].
  phase A (stats):     stream x shard as 16 tiles x 2 half-DMAs, alternating
                       between the sync and scalar HWDGE queues (two physical
                       rings -> closer to the 358 GB/s HBM cap). DVE
                       reduce_sum per half, ScalarE Square+accum_out per
                       tile. One [128,4] fp32 AllReduce across the 8 cores;
                       then scale_c = gamma_c * rsqrt(var_c+eps),
                       shift_c = beta_c - mean_c*scale_c.
  phase B (conv):      x is re-streamed (sync queue; the first images'
                       loads are queued right behind phase A so they prefetch
                       during the AllReduce gap). Per image: ACT computes
                       sign(scale*x + shift) -> fp8e4 into a zero-padded flat
                       [58*58] SBUF plane (3 planes rotating); conv as 9 taps
                       x fp8 DoubleRow matmuls (contracting all 256 ci at
                       once) into [128co x 8x56] PSUM tiles (4-dim moving AP
                       skips the 2 wrap columns entirely); DVE fuses +bias
                       and relu on the PSUM evacuation; DMA out on sync.

sign() outputs +-1 exactly representable in fp8e4, PE accumulates in fp32
(integer sums bounded by 2304), so the conv arithmetic is exact.
"""

import os
import sys

import numpy as np

for _p in ("/opt/trn_rl_repo", "/root/.axon_site/_ro/trn_rl_repo"):
    if os.path.isdir(_p) and _p not in sys.path:
        sys.path.append(_p)

import concourse.bass as bass
import concourse.bacc as bacc
import concourse.tile as tile
from concourse import mybir
from concourse.bass_utils import run_bass_kernel_spmd
from concourse.masks import make_identity

AF = mybir.ActivationFunctionType
ALU = mybir.AluOpType
F32 = mybir.dt.float32
BF16 = mybir.dt.bfloat16
FP8 = mybir.dt.float8e4
DR = mybir.MatmulPerfMode.DoubleRow

N_CORES = 8
N_IMG = 8          # images per core
C = 256            # channels (in == out)
H = W = 56
HW = H * W         # 3136
HALF = HW // 2     # 1568
PW = W + 2         # 58 padded
PLANE = PW * PW    # 3364
# plane data at offset 1 (guard elem before); padded to 3376 so the DoubleRow
# pair stride (PLANE_G fp8 elements) is a multiple of 16
PLANE_G = 3376
EPS = 1e-5
N_TOTAL = 64 * HW  # BN reduction count over full batch
ROWS_PER_BLK = 8
N_BLK = H // ROWS_PER_BLK        # 7
BLK_FREE = ROWS_PER_BLK * PW     # 464 px per matmul (incl. 2 wrap cols/row;
                                 # a flat AP streams ~20% faster than a 4-dim
                                 # AP that skips them: row-transition overhead)
OUT_FREE = ROWS_PER_BLK * W      # 448 valid outputs per block
N_PLANES = 3       # rotating sign planes (sign can run 2 images ahead)

_CACHE = {}


def _build_nc():
    nc = bacc.Bacc(None, target_bir_lowering=False, num_devices=N_CORES)

    x_d = nc.dram_tensor("x", [N_IMG, C, HW], F32, kind="ExternalInput")
    g_d = nc.dram_tensor("gamma", [C], F32, kind="ExternalInput")
    be_d = nc.dram_tensor("beta", [C], F32, kind="ExternalInput")
    w_d = nc.dram_tensor("w", [C, C * 9], F32, kind="ExternalInput")
    b_d = nc.dram_tensor("b", [C], F32, kind="ExternalInput")
    y_d = nc.dram_tensor("y", [N_IMG, C, HW], F32, kind="ExternalOutput")
    # single [128,4] stats AllReduce (ncfw serializes queued collectives and
    # each pays a skew-dominated rendezvous, so splitting per-chunk is a loss)
    cc_in = nc.dram_tensor("cc_in", [128, 4], F32)
    cc_out = nc.dram_tensor("cc_out", [128, 4], F32, addr_space="Shared")
    # tiny warm-up collective: wakes the ncfw collective path early (and eats
    # the first-collective setup cost) so the real AllReduce is fast
    wm_in = nc.dram_tensor("wm_in", [1, 1], F32)
    wm_out = nc.dram_tensor("wm_out", [1, 1], F32, addr_space="Shared")

    with tile.TileContext(nc) as tc:
        with (
            tc.tile_pool(name="persist", bufs=1) as persist,
            tc.tile_pool(name="xin", bufs=10) as xin_pool,     # x staging, both phases
            tc.tile_pool(name="wpre", bufs=1) as wpre_pool,    # w staging
            tc.tile_pool(name="trash", bufs=1) as trash_pool,
            tc.tile_pool(name="outp", bufs=4) as out_pool,
            tc.tile_pool(name="vec", bufs=1) as vec_pool,
        ):
            # padded+binarized activation planes, rotating over 3 buffers so
            # sign() for image n+1/n+2 doesn't WAR-serialize on conv reads:
            # [ci_part, ci_pair(j), guarded flat plane]
            xpads = [
                persist.tile([128, 2, PLANE_G], FP8, name=f"xpad{i}")
                for i in range(N_PLANES)
            ]
            # conv weights, fp8 DoubleRow lhsT layout: [ci_part, tap, co_chunk, j, co]
            wt = persist.tile([128, 9, 2, 2, 128], FP8)

            # ---- tiny setup on the gpsimd queue (order matters: the weight
            # DMAs must reach the queue before the big plane memsets) ----
            wm_sb = vec_pool.tile([1, 1], F32)
            nc.gpsimd.memset(wm_sb, 0.0)
            nc.gpsimd.dma_start(wm_in[:], wm_sb)
            nc.gpsimd.collective_compute(
                "AllReduce",
                ALU.add,
                replica_groups=[list(range(N_CORES))],
                ins=[wm_in[:]],
                outs=[wm_out[:]],
            )

            # per-channel vectors, [128, 2] = (partition, ci_chunk)
            gamma_sb = vec_pool.tile([128, 2], F32)
            beta_sb = vec_pool.tile([128, 2], F32)
            bias_sb = vec_pool.tile([128, 2], F32)
            nc.gpsimd.dma_start(gamma_sb, g_d.rearrange("(c p) -> p c", p=128))
            nc.gpsimd.dma_start(beta_sb, be_d.rearrange("(c p) -> p c", p=128))
            nc.gpsimd.dma_start(bias_sb, b_d.rearrange("(c p) -> p c", p=128))

            # ---------------- phase A x stream: 16 full-tile 1.6MB DMAs, ALL
            # on the sync HWDGE ring, all dispatched up front. One ring with
            # large chunks sustains 300-400 GB/s; splitting across both HWDGE
            # rings makes each drop to ~170 (packet-interleave overhead), and
            # pacing dispatches off compute events strangles the stream.
            N_TILES = 2 * N_IMG  # (n, c) pairs, n-major
            sums = vec_pool.tile([128, 2, N_IMG], F32)       # per tile
            sumsq = vec_pool.tile([128, 2, N_IMG], F32)      # per tile
            cc_sb = vec_pool.tile([128, 2, 2], F32)          # per chunk: (sum, sumsq)

            xa_tiles = []
            for idx in range(N_TILES):
                n, c = divmod(idx, 2)
                xt = xin_pool.tile([128, HW], F32, name=f"xa{idx}", tag="x")
                xa_tiles.append(xt)
                nc.sync.dma_start(xt, x_d[n, c * 128 : (c + 1) * 128, :])

            # ---------------- phase P: weights (own staging pool; scalar-ring
            # DMA + ACT sign + PE transposes + DVE copies, all during the
            # stream; the scalar ring is otherwise idle in phase A)
            ident = vec_pool.tile([128, 128], BF16)
            make_identity(nc, ident)
            ws = wpre_pool.tile([128, 2, C * 9], BF16, bufs=1)
            with tc.tile_pool(name="wps", bufs=2, space="PSUM") as wps:
                for o in range(2):
                    wf = wpre_pool.tile([128, C * 9], F32, bufs=2)
                    nc.scalar.dma_start(wf, w_d[o * 128 : (o + 1) * 128, :])
                    nc.scalar.activation(ws[:, o, :], wf, AF.Sign)
                ws_r = ws.rearrange("p o (ci tap) -> p o ci tap", tap=9)
                for t in range(9):
                    for c in range(2):
                        for o in range(2):
                            pw = wps.tile([128, 128], BF16)
                            nc.tensor.transpose(
                                pw, ws_r[:, o, c * 128 : (c + 1) * 128, t], ident
                            )
                            nc.vector.tensor_copy(wt[:, t, o, c, :], pw)

            # zero the sign planes once (borders + guards stay zero; sign only
            # ever writes the interior) -- on the otherwise idle gpsimd engine
            for xp in xpads:
                nc.gpsimd.memset(xp.rearrange("p a b -> p (a b)"), 0.0)

            # ---------------- phase A reductions (fire per tile as the DMAs
            # land; stats complete ~4us after the last tile arrives)
            for idx in range(N_TILES):
                n, c = divmod(idx, 2)
                xt = xa_tiles[idx]
                nc.vector.reduce_sum(
                    sums[:, c, n : n + 1], xt, axis=mybir.AxisListType.X
                )
                tr = trash_pool.tile([128, HW], F32)
                nc.scalar.activation(
                    tr, xt, AF.Square, accum_out=sumsq[:, c, n : n + 1]
                )

            for c in range(2):
                nc.vector.reduce_sum(
                    cc_sb[:, c, 0:1], sums[:, c, :], axis=mybir.AxisListType.X
                )
                nc.vector.reduce_sum(
                    cc_sb[:, c, 1:2], sumsq[:, c, :], axis=mybir.AxisListType.X
                )

            nc.gpsimd.dma_start(cc_in[:], cc_sb.rearrange("p a b -> p (a b)"))
            nc.gpsimd.collective_compute(
                "AllReduce",
                ALU.add,
                replica_groups=[list(range(N_CORES))],
                ins=[cc_in[:]],
                outs=[cc_out[:]],
            )

            gl = vec_pool.tile([128, 2, 2], F32)
            nc.gpsimd.dma_start(gl.rearrange("p a b -> p (a b)"), cc_out[:])

            # phase-B x reloads go on the scalar ring, gated on the stats
            # being done (the gate DMA reads cc_in, which is written
            # post-consolidation): prefetch fills exactly the AllReduce gap
            # and cannot steal HBM bandwidth from the stats-critical phase A
            # stream. Emitted BEFORE the finalize block so the prefetch
            # dispatches sit ahead of the AR-dependent rstd ops in the ACT
            # queue.
            gate_sb = vec_pool.tile([128, 4], F32)
            nc.scalar.dma_start(gate_sb, cc_in[:])

            xb_tiles = {}

            def emit_xb_dma(n):
                for c in range(2):
                    xt = xin_pool.tile([128, HW], F32, name=f"xb{n}_{c}", tag="x")
                    xb_tiles[(n, c)] = xt
                    nc.scalar.dma_start(xt, x_d[n, c * 128 : (c + 1) * 128, :])

            emit_xb_dma(0)
            emit_xb_dma(1)
            emit_xb_dma(2)

            # per-chunk finalize: scale_c = gamma_c / sqrt(var_c + eps),
            # shift_c = beta_c - mean_c * scale_c. Abs_reciprocal_sqrt's loose
            # precision only scales scl's magnitude (scl stays > 0), which
            # sign() cannot observe — outputs remain exact.
            eps_sb = vec_pool.tile([128, 1], F32)
            nc.vector.memset(eps_sb, EPS)
            mean = vec_pool.tile([128, 2], F32)
            m2 = vec_pool.tile([128, 2], F32)
            var = vec_pool.tile([128, 2], F32)
            rstd = vec_pool.tile([128, 2], F32)
            scl = vec_pool.tile([128, 2], F32)
            sh = vec_pool.tile([128, 2], F32)
            nc.vector.tensor_scalar_mul(mean, gl[:, :, 0], 1.0 / N_TOTAL)
            nc.vector.tensor_tensor(m2, mean, mean, op=ALU.mult)
            nc.vector.scalar_tensor_tensor(
                out=var,
                in0=gl[:, :, 1],
                scalar=1.0 / N_TOTAL,
                in1=m2,
                op0=ALU.mult,
                op1=ALU.subtract,
            )
            nc.scalar.activation(
                rstd, var, AF.Abs_reciprocal_sqrt, bias=eps_sb[:]
            )
            nc.vector.tensor_mul(scl, gamma_sb, rstd)
            nc.vector.tensor_mul(sh, mean, scl)
            nc.vector.tensor_sub(sh, beta_sb, sh)

            # ---------------- phase B: sign + conv ----------------
            def emit_sign(n):
                xp = xpads[n % N_PLANES]
                xrow = xp[:, :, 1 : 1 + PLANE].rearrange(
                    "p j (r w) -> p j r w", w=PW
                )
                for c in range(2):
                    nc.scalar.activation(
                        xrow[:, c, 1 : H + 1, 1 : W + 1],
                        xb_tiles.pop((n, c)).rearrange("p (h w) -> p h w", w=W),
                        AF.Sign,
                        bias=sh[:, c : c + 1],
                        scale=scl[:, c : c + 1],
                    )

            def emit_conv(n, cps):
                xp = xpads[n % N_PLANES]
                for o in range(2):
                    for bi in range(N_BLK):
                        ps = cps.tile([128, BLK_FREE], F32)
                        r0 = bi * ROWS_PER_BLK
                        for t in range(9):
                            ky, kx = divmod(t, 3)
                            s = 1 + (r0 + ky) * PW + (kx - 1)
                            nc.tensor.matmul(
                                ps,
                                wt[:, t, o],
                                xp[:, :, s : s + BLK_FREE],
                                start=(t == 0),
                                stop=(t == 8),
                                perf_mode=DR,
                            )
                        ob = out_pool.tile([128, OUT_FREE], F32)
                        # relu(psum + bias): (x + b) then max(.., 0) on DVE,
                        # dropping the 2 wrap columns of each row
                        nc.vector.tensor_scalar(
                            out=ob,
                            in0=ps.rearrange("p (r c) -> p r c", c=PW)[
                                :, :, 1 : W + 1
                            ],
                            scalar1=bias_sb[:, o : o + 1],
                            scalar2=0.0,
                            op0=ALU.add,
                            op1=ALU.max,
                        )
                        nc.sync.dma_start(
                            y_d[
                                n, o * 128 : (o + 1) * 128,
                                bi * OUT_FREE : (bi + 1) * OUT_FREE,
                            ],
                            ob,
                        )

            with tc.tile_pool(name="cps", bufs=8, space="PSUM") as cps:
                for n in range(N_IMG):
                    emit_sign(n)
                    if n + 3 < N_IMG:
                        emit_xb_dma(n + 3)
                    if n >= 1:
                        emit_conv(n - 1, cps)
                emit_conv(N_IMG - 1, cps)

    nc.finalize()
    return nc


def get_nc():
    if "nc" not in _CACHE:
        _CACHE["nc"] = _build_nc()
    return _CACHE["nc"]


def run(x, gamma, beta, w, b, trace=False, trace_cores=None):
    x = np.ascontiguousarray(np.asarray(x, dtype=np.float32))
    gamma = np.ascontiguousarray(np.asarray(gamma, dtype=np.float32))
    beta = np.ascontiguousarray(np.asarray(beta, dtype=np.float32))
    w = np.ascontiguousarray(np.asarray(w, dtype=np.float32)).reshape(C, C * 9)
    b = np.ascontiguousarray(np.asarray(b, dtype=np.float32))

    nc = get_nc()
    in_maps = []
    for i in range(N_CORES):
        in_maps.append(
            {
                "x": np.ascontiguousarray(
                    x[i * N_IMG : (i + 1) * N_IMG].reshape(N_IMG, C, HW)
                ),
                "gamma": gamma,
                "beta": beta,
                "w": w,
                "b": b,
            }
        )
    res = run_bass_kernel_spmd(
        nc, in_maps, list(range(N_CORES)), trace=trace, trace_cores=trace_cores
    )
    y = np.concatenate(
        [r["y"].reshape(N_IMG, C, H, W) for r in res.results], axis=0
    )
    return y.astype(np.float32), res


def kernel(x, gamma, beta, w, b):
    y, _ = run(x, gamma, beta, w, b, trace=False)
    return y
